# revision 3
# baseline (speedup 1.0000x reference)
# Trainium2 Bass kernel for nn_MemoryReader (retrieval_knn).
#
# Reference math (per batch b):
#   mkf = mk[b].reshape(CK, N)          N = T*H*W = 8192, CK = 64
#   qkf = qk[b].reshape(CK, M)          M = H*W = 1024
#   affinity = softmax_n((2*mkf^T@qkf - |mkf|^2) / sqrt(CK))    # [N, M]
#   out[b]   = mv[b].reshape(CV, N) @ affinity                  # [CV, M], CV=512
#
# Sharding: 8 cores = 4 batches x 2 query-halves (M=1024 -> 512 per core).
# The softmax dim (N) stays local per core -> no collectives.
#
# Per-core device program:
#   - mk loaded as [CK, N] plus an extra lhsT row 64 holding a_sq[n] =
#     sum_c mk[c,n]^2 (computed on-chip: ACT square + ones-matmul).
#     qk loaded as [CK, M] plus an extra row of -0.5, so the phase-1 matmul
#     directly produces logits*4 = ab - a_sq/2; ACT exp with scale=0.25
#     gives the unnormalized softmax weights E.
#   - Phase 2: acc[c_tile] += mv_t_tile-slices @ E accumulated over the 64
#     n-tiles in PSUM; a parallel ones-matmul accumulates S[m] = sum_n E.
#   - Epilogue: r = 1/S broadcast across partitions via a K=1 matmul with a
#     ones row; out = acc * r_bcast, DMA to DRAM.
# Matmuls run as float32r (full PE speed at free-dim >= 256); every on-chip
# producer feeding an fp32r matmul writes dtype float32r (BIR verifier rule).

import numpy as np

B = 4
CK = 64
CV = 512
N = 8192  # T*H*W memory tokens
M = 512  # query tokens per core (1024 / 2)
P = 128
NT = N // P  # 64 n-tiles
SQ_CH = 512
NSQ = N // SQ_CH  # 16 chunks for the a_sq prelude
NCT = CV // P  # 4 output c-tiles

_cache = {}


def _build_program():
    import concourse.bacc as bacc
    import concourse.tile as tile
    from concourse import mybir

    f32 = mybir.dt.float32
    f32r = mybir.dt.float32r
    Exp = mybir.ActivationFunctionType.Exp

    nc = bacc.Bacc(
        "TRN2",
        target_bir_lowering=False,
        debug=False,
        num_devices=8,
    )

    mk_d = nc.dram_tensor("mk", [CK, N], f32, kind="ExternalInput").ap()
    qk_d = nc.dram_tensor("qk", [CK, M], f32, kind="ExternalInput").ap()
    mvt_d = nc.dram_tensor("mvt", [N, CV], f32, kind="ExternalInput").ap()
    out_d = nc.dram_tensor("out", [CV, M], f32, kind="ExternalOutput").ap()

    with tile.TileContext(nc) as tc:
        with (
            tc.tile_pool(name="const", bufs=1) as const,
            tc.tile_pool(name="big", bufs=1) as bigp,
            tc.tile_pool(name="sqp", bufs=4) as sqp,
            tc.tile_pool(name="ep", bufs=4) as ep,
            tc.tile_pool(name="mvp", bufs=4) as mvp,
            tc.tile_pool(name="outp", bufs=2) as outp,
            tc.tile_pool(name="pab", bufs=2, space="PSUM") as pab,
            tc.tile_pool(name="pacc", bufs=1, space="PSUM") as pacc,
            tc.tile_pool(name="ps", bufs=1, space="PSUM") as ps,
        ):
            ones_f = const.tile([P, 1], f32, tag="ones_f")
            nc.vector.memset(ones_f[:], 1.0)
            ones = const.tile([P, 1], f32r, tag="ones")
            nc.vector.tensor_copy(ones[:], ones_f[:])
            ones_r = const.tile([1, P], f32, tag="ones_r")
            nc.vector.memset(ones_r[:], 1.0)

            neg_half = const.tile([1, M], f32, tag="neg_half")
            nc.vector.memset(neg_half[:], -0.5)
            qk_sb = const.tile([CK + 1, M], f32r, tag="qk")
            nc.sync.dma_start(qk_sb[0:CK, :], qk_d[:, :].bitcast(f32r))
            nc.vector.tensor_copy(qk_sb[CK : CK + 1, :], neg_half[:])

            # mk plus the a_sq augmentation row
            mk_sb = bigp.tile([CK + 1, N], f32r, tag="mk")
            for j in range(4):
                nc.sync.dma_start(
                    mk_sb[0:CK, j * 2048 : (j + 1) * 2048],
                    mk_d[:, j * 2048 : (j + 1) * 2048].bitcast(f32r),
                )

            # a_sq[n] = sum_c mk[c,n]^2 into mk_sb row CK
            for j in range(NSQ):
                sl = slice(j * SQ_CH, (j + 1) * SQ_CH)
                sq_t = sqp.tile([CK, SQ_CH], f32r, tag="sq")
                nc.scalar.square(sq_t[:], mk_sb[0:CK, sl])
                asq_ps = pab.tile([1, SQ_CH], f32, tag="ab")
                nc.tensor.matmul(
                    asq_ps[:],
                    ones[0:CK, :],
                    sq_t[:],
                    start=True,
                    stop=True,
                )
                nc.vector.tensor_copy(mk_sb[CK : CK + 1, sl], asq_ps[:])

            # persistent PSUM accumulators
            acc = [pacc.tile([P, M], f32, tag=f"acc{ct}", name=f"acc{ct}") for ct in range(NCT)]
            s_ps = ps.tile([1, M], f32, tag="s")

            for i in range(NT):
                nsl = slice(i * P, (i + 1) * P)
                first, last = (i == 0), (i == NT - 1)

                ab = pab.tile([P, M], f32, tag="ab", name="ab")
                nc.tensor.matmul(
                    ab[:],
                    mk_sb[:, nsl],
                    qk_sb[:],
                    start=True,
                    stop=True,
                )
                e_t = ep.tile([P, M], f32r, tag="e", name="e_t")
                nc.scalar.activation(e_t[:], ab[:], Exp, scale=0.25)

                mv_t = mvp.tile([P, CV], f32r, tag="mv", name="mv_t")
                nc.sync.dma_start(mv_t[:], mvt_d[nsl, :].bitcast(f32r))

                for ct in range(NCT):
                    nc.tensor.matmul(
                        acc[ct][:],
                        mv_t[:, ct * P : (ct + 1) * P],
                        e_t[:],
                        start=first,
                        stop=last,
                    )
                nc.tensor.matmul(
                    s_ps[:],
                    ones[:],
                    e_t[:],
                    start=first,
                    stop=last,
                )

            # epilogue: normalize by S
            s_row = const.tile([1, M], f32, tag="srow")
            nc.vector.tensor_copy(s_row[:], s_ps[:])
            r_row = const.tile([1, M], f32, tag="rrow")
            nc.vector.reciprocal(r_row[:], s_row[:])
            bc_ps = pab.tile([P, M], f32, tag="ab", name="bc_ps")
            # plain fp32 matmul (exact); K=1 broadcast of r across partitions
            nc.tensor.matmul(bc_ps[:], ones_r[:], r_row[:], start=True, stop=True)
            bc_sb = const.tile([P, M], f32, tag="bc")
            nc.vector.tensor_copy(bc_sb[:], bc_ps[:])
            for ct in range(NCT):
                o_t = outp.tile([P, M], f32, tag="o", name="o_t")
                nc.vector.tensor_mul(o_t[:], acc[ct][:], bc_sb[:])
                nc.sync.dma_start(out_d[ct * P : (ct + 1) * P, :], o_t[:])

    nc.compile()
    return nc


def _get_program():
    if "nc" not in _cache:
        _cache["nc"] = _build_program()
    return _cache["nc"]


def kernel(mk, qk, mv):
    from concourse import bass_utils

    nc = _get_program()

    in_maps = []
    for b in range(B):
        mkf = np.ascontiguousarray(mk[b].reshape(CK, N), dtype=np.float32)
        qkf = qk[b].reshape(CK, 2 * M)
        mvt = np.ascontiguousarray(
            mv[b].reshape(CV, N).T, dtype=np.float32
        )
        for h in range(2):
            in_maps.append(
                {
                    "mk": mkf,
                    "qk": np.ascontiguousarray(qkf[:, h * M : (h + 1) * M]),
                    "mvt": mvt,
                }
            )

    res = bass_utils.run_bass_kernel_spmd(nc, in_maps, core_ids=list(range(8)))
    kernel._last_results = res

    outs = [r["out"] for r in res.results]
    full = np.empty((B, CV, 2 * M), dtype=np.float32)
    for b in range(B):
        full[b, :, :M] = outs[2 * b]
        full[b, :, M:] = outs[2 * b + 1]
    return full.reshape(B, CV, 32, 32)
